# revision 5
# baseline (speedup 1.0000x reference)
"""Doc2vec-style embedding lookup + negative-sampling scores on 8 trn2 cores.

reference:
    x[b, :] = D[doc_ids[b]] + sum_c W[context_ids[b, c]]      # (B, 256)
    scores[b, k] = dot(x[b], O[:, target_noise_ids[b, k]])    # (B, 6)

Strategy (v2): data-parallel over batch (512 items/core), tables replicated
in bf16.  The baseline issued 60 indirect DMAs per core; each costs ~1.4us
of serialized Q7 SWDGE descriptor generation (994ns fixed + 0.34ns/desc),
so the kernel was Q7-bound at ~102us.  This version uses InstDMAGatherAnt
(dma_gather, mlp library), which amortizes the 994ns fixed cost over
thousands of descriptors.

dma_gather indices are int16 (sign-extended by the Q7; negatives fatal
mid-list), so a single gather can only span 32768 rows.  W (50000 rows) and
O^T (50000) are split into two 25001-row windows, each ending in an all-zero
row.  Every item gets 8 ctx slots in BOTH windows: real (window-relative)
ids fill slots in the window that owns them, remaining slots point at the
zero row.  Summing all 16 gathered rows equals the real 8-row ctx sum
(ctx slots are exchangeable under +).  Noise cols use the same trick per
slot: col = lo_gather[k] + hi_gather[k] (one is the real row, one is zero).
Doc rows (100000, int32 indices) stay on the exact indirect-DMA path.

Engines: SWDGE gathers (GpSimd) -> DVE folds lo+hi, 8-slot reduce, doc add,
noise mult -> ACT does the 6 per-slot dot-product accumulations
(activation accum_out = per-partition sum over free dim).
"""

import sys

sys.path.insert(0, "/opt/trn_rl_repo")

from contextlib import ExitStack

import ml_dtypes
import numpy as np

from concourse import bacc, bass, mybir
from concourse.bass_utils import run_bass_kernel_spmd
from concourse.library_config import mlp

VEC = 256
N_DOCS = 100000
N_WORDS = 50000
B = 4096
N_CTX = 8
N_NOISE = 6
N_CORES = 8
BPC = B // N_CORES  # 512
P = 128
TILES = BPC // P  # 4
WIN = 25000  # rows per gather window (zero row at local index WIN)
WROWS = WIN + 1
# table row layout (bf16, 256 wide)
W_LO = N_DOCS
W_HI = W_LO + WROWS
O_LO = W_HI + WROWS
O_HI = O_LO + WROWS
T_ROWS = O_HI + WROWS  # 200004

BF16 = mybir.dt.bfloat16

# idxg column layout (int16, 16-wrapped, replicated x8):
# cols per segment = num_idxs // 16
SEG = {}
_c = 0
for _name, _n in [
    ("wlo0", 2048), ("whi0", 2048), ("olo0", 1536), ("ohi0", 1536),
    ("wlo1", 2048), ("whi1", 2048), ("olo1", 1536), ("ohi1", 1536),
]:
    SEG[_name] = (_c, _n)
    _c += _n // 16
IDX_COLS = _c  # 896

_nc_cache = None


def build_nc():
    nc = bacc.Bacc(None, target_bir_lowering=False, debug=False, num_swdge_queues=1, dynamic_dma_scratch_size=65536)
    tbl = nc.declare_dram_parameter("tbl", [T_ROWS, VEC], BF16, isOutput=False)
    idxg = nc.declare_dram_parameter("idxg", [P, IDX_COLS], mybir.dt.int16, isOutput=False)
    idxd = nc.declare_dram_parameter("idxd", [P, TILES], mybir.dt.int32, isOutput=False)
    out = nc.declare_dram_parameter("out", [P, TILES * N_NOISE], mybir.dt.float32, isOutput=True)

    with ExitStack() as ctx:
        block = ctx.enter_context(nc.Block(no_gpsimd_drain=True))
        sem_idx = ctx.enter_context(nc.semaphore("sem_idx"))
        semW = [ctx.enter_context(nc.semaphore(f"semW{i}")) for i in range(2)]
        semO = [ctx.enter_context(nc.semaphore(f"semO{i}")) for i in range(2)]
        semD = ctx.enter_context(nc.semaphore("semD"))
        sem_prod = ctx.enter_context(nc.semaphore("sem_prod"))
        sem_act = ctx.enter_context(nc.semaphore("sem_act"))
        sem_out = ctx.enter_context(nc.semaphore("sem_out"))

        idxg_t = ctx.enter_context(nc.sbuf_tensor("idxg_t", [P, IDX_COLS], mybir.dt.int16))
        idxd_t = ctx.enter_context(nc.sbuf_tensor("idxd_t", [P, TILES], mybir.dt.int32))
        bufW = ctx.enter_context(nc.sbuf_tensor("bufW", [P, 64 * VEC], BF16))
        bufO = ctx.enter_context(nc.sbuf_tensor("bufO", [P, 48 * VEC], BF16))
        bufD = ctx.enter_context(nc.sbuf_tensor("bufD", [P, TILES * VEC], BF16))
        w8 = ctx.enter_context(nc.sbuf_tensor("w8", [P, 8 * VEC], BF16))
        xa = ctx.enter_context(nc.sbuf_tensor("xa", [P, VEC], BF16))
        xb = ctx.enter_context(nc.sbuf_tensor("xb", [P, VEC], BF16))
        cols = ctx.enter_context(nc.sbuf_tensor("cols", [P, N_NOISE * VEC], BF16))
        prod2 = ctx.enter_context(nc.sbuf_tensor("prod2", [P, 2 * N_NOISE * VEC], BF16))
        dump = ctx.enter_context(nc.sbuf_tensor("dump", [P, VEC], BF16))
        score_t = ctx.enter_context(nc.sbuf_tensor("score_t", [P, TILES * N_NOISE], mybir.dt.float32))

        # bufW slots: [pair*32 + 0:16) = lo (t_even s0-7, t_odd s0-7), +16 = hi
        # bufO slots: [pair*24 + 0:12) = lo (t_even k0-5, t_odd k0-5), +12 = hi

        @block.sync
        def _(s: bass.BassEngine):
            s.dma_start(out=idxg_t[:, :], in_=idxg[:, :]).then_inc(sem_idx, 16)
            s.dma_start(out=idxd_t[:, :], in_=idxd[:, :]).then_inc(sem_idx, 16)
            s.wait_ge(sem_act, TILES)
            s.dma_start(out=out[:, :], in_=score_t[:, :]).then_inc(sem_out, 16)
            s.wait_ge(sem_out, 16)

        @block.gpsimd
        def _(g: bass.BassGpSimd):
            g.load_library(mlp)
            g.wait_ge(sem_idx, 32)

            def gather(seg, base, buf, slot0, nslots, sem):
                c0, n = SEG[seg]
                nchunks = (n + 1023) // 1024
                step = n // nchunks
                for ci in range(nchunks):
                    sl0 = slot0 + ci * (step // 128)
                    g.dma_gather(
                        out_ap=buf[:, sl0 * VEC : (sl0 + step // 128) * VEC].rearrange(
                            "p (j d) -> p j d", j=step // 128
                        ),
                        in_ap=tbl[base : base + WROWS, :],
                        idxs_ap=idxg_t[:, c0 + ci * step // 16 : c0 + (ci + 1) * step // 16],
                        num_idxs=step,
                        num_idxs_reg=step,
                        elem_size=VEC,
                        single_packet=True,
                    ).then_inc(sem, 16)

            for pair in range(2):
                sfx = str(pair)
                gather("wlo" + sfx, W_LO, bufW, pair * 32, 16, semW[pair])
                gather("whi" + sfx, W_HI, bufW, pair * 32 + 16, 16, semW[pair])
                for t in (2 * pair, 2 * pair + 1):
                    g.indirect_dma_start(
                        out=bufD[:, t * VEC : (t + 1) * VEC],
                        out_offset=None,
                        in_=tbl[:],
                        in_offset=bass.IndirectOffsetOnAxis(
                            ap=idxd_t[:, t : t + 1], axis=0
                        ),
                    ).then_inc(semD, 16)
                gather("olo" + sfx, O_LO, bufO, pair * 24, 12, semO[pair])
                gather("ohi" + sfx, O_HI, bufO, pair * 24 + 12, 12, semO[pair])

        @block.vector
        def _(v: bass.BassVectorEngine):
            with nc.allow_low_precision(reason="bf16 x/prod intermediates, f32 final accum"):
                for t in range(TILES):
                    pair, par = t // 2, t % 2
                    wlo0 = (pair * 32 + par * 8) * VEC
                    whi0 = (pair * 32 + 16 + par * 8) * VEC
                    v.wait_ge(semW[pair], 64)
                    v.tensor_tensor(
                        out=w8[:, :],
                        in0=bufW[:, wlo0 : wlo0 + 8 * VEC],
                        in1=bufW[:, whi0 : whi0 + 8 * VEC],
                        op=mybir.AluOpType.add,
                    )
                    v.tensor_reduce(
                        out=xa[:, :],
                        in_=w8[:, :].rearrange("p (s d) -> p d s", s=8),
                        axis=mybir.AxisListType.X,
                        op=mybir.AluOpType.add,
                    )
                    v.wait_ge(semD, 16 * (t + 1))
                    v.tensor_tensor(
                        out=xb[:, :],
                        in0=xa[:, :],
                        in1=bufD[:, t * VEC : (t + 1) * VEC],
                        op=mybir.AluOpType.add,
                    )
                    olo0 = (pair * 24 + par * 6) * VEC
                    ohi0 = (pair * 24 + 12 + par * 6) * VEC
                    v.wait_ge(semO[pair], 64)
                    if t >= 2:
                        v.wait_ge(sem_act, t - 1)  # prod2 slot t%2 free
                    v.tensor_tensor(
                        out=cols[:, :],
                        in0=bufO[:, olo0 : olo0 + 6 * VEC],
                        in1=bufO[:, ohi0 : ohi0 + 6 * VEC],
                        op=mybir.AluOpType.add,
                    )
                    pr = prod2[:, par * 6 * VEC : (par + 1) * 6 * VEC]
                    v.tensor_tensor(
                        out=pr.rearrange("p (k d) -> p k d", k=N_NOISE),
                        in0=xb[:, None, :].to_broadcast([P, N_NOISE, VEC]),
                        in1=cols[:, :].rearrange("p (k d) -> p k d", k=N_NOISE),
                        op=mybir.AluOpType.mult,
                    ).then_inc(sem_prod, 1)

        @block.scalar
        def _(a: bass.BassScalarEngine):
            for t in range(TILES):
                par = t % 2
                a.wait_ge(sem_prod, t + 1)
                for k in range(N_NOISE):
                    ins = a.activation(
                        out=dump[:, :],
                        in_=prod2[:, (par * 6 + k) * VEC : (par * 6 + k + 1) * VEC],
                        func=mybir.ActivationFunctionType.Copy,
                        accum_out=score_t[:, t * N_NOISE + k : t * N_NOISE + k + 1],
                    )
                ins.then_inc(sem_act, 1)

    nc.compile()
    return nc


def get_nc():
    global _nc_cache
    if _nc_cache is None:
        _nc_cache = build_nc()
    return _nc_cache


def make_table(D, W, O):
    """bf16 table [200004, 256]: D; Wlo; z; Whi; z; Olo; z; Ohi; z."""
    bf = ml_dtypes.bfloat16
    tbl = np.zeros((T_ROWS, VEC), dtype=bf)
    tbl[:N_DOCS] = np.asarray(D, np.float32).astype(bf)
    Wb = np.asarray(W, np.float32).astype(bf)
    tbl[W_LO : W_LO + WIN] = Wb[:WIN]
    tbl[W_HI : W_HI + WIN] = Wb[WIN:]
    Ob = np.ascontiguousarray(np.asarray(O, np.float32).T).astype(bf)
    tbl[O_LO : O_LO + WIN] = Ob[:WIN]
    tbl[O_HI : O_HI + WIN] = Ob[WIN:]
    return tbl


def pack_idx(L):
    """list of n int idxs -> [128, n//16] int16 (16-wrapped, replicated x8)."""
    n = L.shape[0]
    A = L.reshape(n // 16, 16).T.astype(np.int16)  # [16, n//16]
    return np.tile(A, (8, 1))


def make_core_inputs(context_ids, doc_ids, target_noise_ids, core):
    """Returns (idxg [128, IDX_COLS] i16, idxd [128, TILES] i32)."""
    sl = slice(core * BPC, (core + 1) * BPC)
    ctx = np.asarray(context_ids, np.int64)[sl].reshape(TILES, P, N_CTX)
    doc = np.asarray(doc_ids, np.int64)[sl].reshape(TILES, P)
    noi = np.asarray(target_noise_ids, np.int64)[sl].reshape(TILES, P, N_NOISE)

    idxd = doc.T.astype(np.int32).copy()  # [128, TILES]

    BIG = 1 << 20
    lo = np.sort(np.where(ctx < WIN, ctx, BIG), axis=-1)
    lo = np.where(lo >= BIG, WIN, lo)  # [T, P, 8]
    hi = np.sort(np.where(ctx >= WIN, ctx - WIN, BIG), axis=-1)
    hi = np.where(hi >= BIG, WIN, hi)
    nlo = np.where(noi < WIN, noi, WIN)  # [T, P, 6]
    nhi = np.where(noi >= WIN, noi - WIN, WIN)

    idxg = np.empty((P, IDX_COLS), dtype=np.int16)
    for pair in range(2):
        tt = slice(2 * pair, 2 * pair + 2)
        for name, arr in [("wlo", lo), ("whi", hi), ("olo", nlo), ("ohi", nhi)]:
            # positions i = p + 128*(nslots*t_local + s) -> L[(t*ns + s)*128 + p]
            L = arr[tt].transpose(0, 2, 1).reshape(-1)  # [(t s p)]
            c0, n = SEG[name + str(pair)]
            idxg[:, c0 : c0 + n // 16] = pack_idx(L)
    return idxg, idxd


def unshard_output(outs):
    parts = []
    for o in outs:
        parts.append(
            np.asarray(o, np.float32)
            .reshape(P, TILES, N_NOISE)
            .transpose(1, 0, 2)
            .reshape(BPC, N_NOISE)
        )
    return np.concatenate(parts, axis=0)


def _install_profile_hook():
    import types

    if "antenv.axon_hooks" in sys.modules:
        return
    import antenv
    from trn_agent_boot.trn_boot import _ntff_profile_via_ctypes

    mod = types.ModuleType("antenv.axon_hooks")
    _state = {"hook": _ntff_profile_via_ctypes("/opt/axon/libaxon_pjrt.so")}
    mod.set_axon_ntff_profile_hook = lambda h: _state.__setitem__("hook", h)
    mod.get_axon_ntff_profile_hook = lambda: _state["hook"]
    sys.modules["antenv.axon_hooks"] = mod
    antenv.axon_hooks = mod


def kernel(context_ids, doc_ids, target_noise_ids, D, W, O, _trace=False):
    if _trace:
        _install_profile_hook()
    nc = get_nc()
    tbl = make_table(D, W, O)
    in_maps = []
    for c in range(N_CORES):
        idxg, idxd = make_core_inputs(context_ids, doc_ids, target_noise_ids, c)
        in_maps.append({"tbl": tbl, "idxg": idxg, "idxd": idxd})
    res = run_bass_kernel_spmd(
        nc, in_maps, core_ids=list(range(N_CORES)), trace=_trace
    )
    scores = unshard_output([res.results[c]["out"] for c in range(N_CORES)])
    if _trace:
        kernel.last_exec_time_ns = res.exec_time_ns
        kernel.last_results = res
    return scores


# revision 6
# speedup vs baseline: 1.2650x; 1.2650x over previous
"""Doc2vec-style embedding lookup + negative-sampling scores on 8 trn2 cores.

reference:
    x[b, :] = D[doc_ids[b]] + sum_c W[context_ids[b, c]]      # (B, 256)
    scores[b, k] = dot(x[b], O[:, target_noise_ids[b, k]])    # (B, 6)

Strategy (v2): data-parallel over batch (512 items/core), tables replicated
in bf16.  The baseline issued 60 indirect DMAs per core; each costs ~1.4us
of serialized Q7 SWDGE descriptor generation (994ns fixed + 0.34ns/desc),
so the kernel was Q7-bound at ~102us.  This version uses InstDMAGatherAnt
(dma_gather, mlp library), which amortizes the 994ns fixed cost over
thousands of descriptors.

dma_gather indices are int16 (sign-extended by the Q7; negatives fatal
mid-list), so a single gather can only span 32768 rows.  W (50000 rows) and
O^T (50000) are split into two 25001-row windows, each ending in an all-zero
row.  Every item gets 8 ctx slots in BOTH windows: real (window-relative)
ids fill slots in the window that owns them, remaining slots point at the
zero row.  Summing all 16 gathered rows equals the real 8-row ctx sum
(ctx slots are exchangeable under +).  Noise cols use the same trick per
slot: col = lo_gather[k] + hi_gather[k] (one is the real row, one is zero).
Doc rows (100000, int32 indices) stay on the exact indirect-DMA path.

Engines: SWDGE gathers (GpSimd) -> DVE folds lo+hi, 8-slot reduce, doc add,
noise mult -> ACT does the 6 per-slot dot-product accumulations
(activation accum_out = per-partition sum over free dim).
"""

import sys

sys.path.insert(0, "/opt/trn_rl_repo")

from contextlib import ExitStack

import ml_dtypes
import numpy as np

from concourse import bacc, bass, mybir
from concourse.bass_utils import run_bass_kernel_spmd
from concourse.library_config import mlp

VEC = 256
N_DOCS = 100000
N_WORDS = 50000
B = 4096
N_CTX = 8
N_NOISE = 6
N_CORES = 8
BPC = B // N_CORES  # 512
P = 128
TILES = BPC // P  # 4
WIN = 25000  # rows per gather window (zero row at local index WIN)
WROWS = WIN + 1
# table row layout (bf16, 256 wide)
W_LO = N_DOCS
W_HI = W_LO + WROWS
O_LO = W_HI + WROWS
O_HI = O_LO + WROWS
T_ROWS = O_HI + WROWS  # 200004

BF16 = mybir.dt.bfloat16

# idxg column layout (int16, 16-wrapped, replicated x8):
# cols per segment = num_idxs // 16
SEG = {}
_c = 0
for _name, _n in [
    ("wlo0", 2048), ("whi0", 2048), ("olo0", 1536), ("ohi0", 1536),
    ("wlo1", 2048), ("whi1", 2048), ("olo1", 1536), ("ohi1", 1536),
]:
    SEG[_name] = (_c, _n)
    _c += _n // 16
IDX_COLS = _c  # 896

_nc_cache = None


def build_nc():
    nc = bacc.Bacc(None, target_bir_lowering=False, debug=False, num_swdge_queues=2, dynamic_dma_scratch_size=65536)
    tbl = nc.declare_dram_parameter("tbl", [T_ROWS, VEC], BF16, isOutput=False)
    idxg = nc.declare_dram_parameter("idxg", [P, IDX_COLS], mybir.dt.int16, isOutput=False)
    idxd = nc.declare_dram_parameter("idxd", [P, TILES], mybir.dt.int32, isOutput=False)
    out = nc.declare_dram_parameter("out", [P, TILES * N_NOISE], mybir.dt.float32, isOutput=True)

    with ExitStack() as ctx:
        block = ctx.enter_context(nc.Block(no_gpsimd_drain=True))
        sem_idx = ctx.enter_context(nc.semaphore("sem_idx"))
        semW = [ctx.enter_context(nc.semaphore(f"semW{i}")) for i in range(2)]
        semO = [ctx.enter_context(nc.semaphore(f"semO{i}")) for i in range(2)]
        semD = ctx.enter_context(nc.semaphore("semD"))
        sem_prod = ctx.enter_context(nc.semaphore("sem_prod"))
        sem_act = ctx.enter_context(nc.semaphore("sem_act"))
        sem_out = ctx.enter_context(nc.semaphore("sem_out"))

        idxg_t = ctx.enter_context(nc.sbuf_tensor("idxg_t", [P, IDX_COLS], mybir.dt.int16))
        idxd_t = ctx.enter_context(nc.sbuf_tensor("idxd_t", [P, TILES], mybir.dt.int32))
        bufW = ctx.enter_context(nc.sbuf_tensor("bufW", [P, 64 * VEC], BF16))
        bufO = ctx.enter_context(nc.sbuf_tensor("bufO", [P, 48 * VEC], BF16))
        bufD = ctx.enter_context(nc.sbuf_tensor("bufD", [P, TILES * VEC], BF16))
        w8 = ctx.enter_context(nc.sbuf_tensor("w8", [P, 8 * VEC], BF16))
        xa = ctx.enter_context(nc.sbuf_tensor("xa", [P, VEC], BF16))
        xb = ctx.enter_context(nc.sbuf_tensor("xb", [P, VEC], BF16))
        cols = ctx.enter_context(nc.sbuf_tensor("cols", [P, N_NOISE * VEC], BF16))
        prod2 = ctx.enter_context(nc.sbuf_tensor("prod2", [P, 2 * N_NOISE * VEC], BF16))
        dump = ctx.enter_context(nc.sbuf_tensor("dump", [P, VEC], BF16))
        score_t = ctx.enter_context(nc.sbuf_tensor("score_t", [P, TILES * N_NOISE], mybir.dt.float32))

        # bufW slots: [pair*32 + 0:16) = lo (t_even s0-7, t_odd s0-7), +16 = hi
        # bufO slots: [pair*24 + 0:12) = lo (t_even k0-5, t_odd k0-5), +12 = hi

        @block.sync
        def _(s: bass.BassEngine):
            s.dma_start(out=idxg_t[:, :], in_=idxg[:, :]).then_inc(sem_idx, 16)
            s.dma_start(out=idxd_t[:, :], in_=idxd[:, :]).then_inc(sem_idx, 16)
            s.wait_ge(sem_act, TILES)
            s.dma_start(out=out[:, :], in_=score_t[:, :]).then_inc(sem_out, 16)
            s.wait_ge(sem_out, 16)

        @block.gpsimd
        def _(g: bass.BassGpSimd):
            g.load_library(mlp)
            g.wait_ge(sem_idx, 32)

            qctr = [0]

            def gather(seg, base, buf, slot0, nslots, sem):
                c0, n = SEG[seg]
                q = qctr[0] % 2
                qctr[0] += 1
                g.dma_gather(
                    out_ap=buf[:, slot0 * VEC : (slot0 + nslots) * VEC].rearrange(
                        "p (j d) -> p j d", j=nslots
                    ),
                    in_ap=tbl[base : base + WROWS, :],
                    idxs_ap=idxg_t[:, c0 : c0 + n // 16],
                    num_idxs=n,
                    num_idxs_reg=n,
                    elem_size=VEC,
                    single_packet=False,
                    queue_num=q,
                ).then_inc(sem, 16)

            for pair in range(2):
                sfx = str(pair)
                gather("wlo" + sfx, W_LO, bufW, pair * 32, 16, semW[pair])
                gather("whi" + sfx, W_HI, bufW, pair * 32 + 16, 16, semW[pair])
                for t in (2 * pair, 2 * pair + 1):
                    g.indirect_dma_start(
                        out=bufD[:, t * VEC : (t + 1) * VEC],
                        out_offset=None,
                        in_=tbl[:],
                        in_offset=bass.IndirectOffsetOnAxis(
                            ap=idxd_t[:, t : t + 1], axis=0
                        ),
                    ).then_inc(semD, 16)
                gather("olo" + sfx, O_LO, bufO, pair * 24, 12, semO[pair])
                gather("ohi" + sfx, O_HI, bufO, pair * 24 + 12, 12, semO[pair])

        @block.vector
        def _(v: bass.BassVectorEngine):
            with nc.allow_low_precision(reason="bf16 x/prod intermediates, f32 final accum"):
                for t in range(TILES):
                    pair, par = t // 2, t % 2
                    wlo0 = (pair * 32 + par * 8) * VEC
                    whi0 = (pair * 32 + 16 + par * 8) * VEC
                    v.wait_ge(semW[pair], 32)
                    v.tensor_tensor(
                        out=w8[:, :],
                        in0=bufW[:, wlo0 : wlo0 + 8 * VEC],
                        in1=bufW[:, whi0 : whi0 + 8 * VEC],
                        op=mybir.AluOpType.add,
                    )
                    v.tensor_reduce(
                        out=xa[:, :],
                        in_=w8[:, :].rearrange("p (s d) -> p d s", s=8),
                        axis=mybir.AxisListType.X,
                        op=mybir.AluOpType.add,
                    )
                    v.wait_ge(semD, 16 * (t + 1))
                    v.tensor_tensor(
                        out=xb[:, :],
                        in0=xa[:, :],
                        in1=bufD[:, t * VEC : (t + 1) * VEC],
                        op=mybir.AluOpType.add,
                    )
                    olo0 = (pair * 24 + par * 6) * VEC
                    ohi0 = (pair * 24 + 12 + par * 6) * VEC
                    v.wait_ge(semO[pair], 32)
                    if t >= 2:
                        v.wait_ge(sem_act, t - 1)  # prod2 slot t%2 free
                    v.tensor_tensor(
                        out=cols[:, :],
                        in0=bufO[:, olo0 : olo0 + 6 * VEC],
                        in1=bufO[:, ohi0 : ohi0 + 6 * VEC],
                        op=mybir.AluOpType.add,
                    )
                    pr = prod2[:, par * 6 * VEC : (par + 1) * 6 * VEC]
                    v.tensor_tensor(
                        out=pr.rearrange("p (k d) -> p k d", k=N_NOISE),
                        in0=xb[:, None, :].to_broadcast([P, N_NOISE, VEC]),
                        in1=cols[:, :].rearrange("p (k d) -> p k d", k=N_NOISE),
                        op=mybir.AluOpType.mult,
                    ).then_inc(sem_prod, 1)

        @block.scalar
        def _(a: bass.BassScalarEngine):
            for t in range(TILES):
                par = t % 2
                a.wait_ge(sem_prod, t + 1)
                for k in range(N_NOISE):
                    ins = a.activation(
                        out=dump[:, :],
                        in_=prod2[:, (par * 6 + k) * VEC : (par * 6 + k + 1) * VEC],
                        func=mybir.ActivationFunctionType.Copy,
                        accum_out=score_t[:, t * N_NOISE + k : t * N_NOISE + k + 1],
                    )
                ins.then_inc(sem_act, 1)

    nc.compile()
    return nc


def get_nc():
    global _nc_cache
    if _nc_cache is None:
        _nc_cache = build_nc()
    return _nc_cache


def make_table(D, W, O):
    """bf16 table [200004, 256]: D; Wlo; z; Whi; z; Olo; z; Ohi; z."""
    bf = ml_dtypes.bfloat16
    tbl = np.zeros((T_ROWS, VEC), dtype=bf)
    tbl[:N_DOCS] = np.asarray(D, np.float32).astype(bf)
    Wb = np.asarray(W, np.float32).astype(bf)
    tbl[W_LO : W_LO + WIN] = Wb[:WIN]
    tbl[W_HI : W_HI + WIN] = Wb[WIN:]
    Ob = np.ascontiguousarray(np.asarray(O, np.float32).T).astype(bf)
    tbl[O_LO : O_LO + WIN] = Ob[:WIN]
    tbl[O_HI : O_HI + WIN] = Ob[WIN:]
    return tbl


def pack_idx(L):
    """list of n int idxs -> [128, n//16] int16 (16-wrapped, replicated x8)."""
    n = L.shape[0]
    A = L.reshape(n // 16, 16).T.astype(np.int16)  # [16, n//16]
    return np.tile(A, (8, 1))


def make_core_inputs(context_ids, doc_ids, target_noise_ids, core):
    """Returns (idxg [128, IDX_COLS] i16, idxd [128, TILES] i32)."""
    sl = slice(core * BPC, (core + 1) * BPC)
    ctx = np.asarray(context_ids, np.int64)[sl].reshape(TILES, P, N_CTX)
    doc = np.asarray(doc_ids, np.int64)[sl].reshape(TILES, P)
    noi = np.asarray(target_noise_ids, np.int64)[sl].reshape(TILES, P, N_NOISE)

    idxd = doc.T.astype(np.int32).copy()  # [128, TILES]

    BIG = 1 << 20
    lo = np.sort(np.where(ctx < WIN, ctx, BIG), axis=-1)
    lo = np.where(lo >= BIG, WIN, lo)  # [T, P, 8]
    hi = np.sort(np.where(ctx >= WIN, ctx - WIN, BIG), axis=-1)
    hi = np.where(hi >= BIG, WIN, hi)
    nlo = np.where(noi < WIN, noi, WIN)  # [T, P, 6]
    nhi = np.where(noi >= WIN, noi - WIN, WIN)

    idxg = np.empty((P, IDX_COLS), dtype=np.int16)
    for pair in range(2):
        tt = slice(2 * pair, 2 * pair + 2)
        for name, arr in [("wlo", lo), ("whi", hi), ("olo", nlo), ("ohi", nhi)]:
            # positions i = p + 128*(nslots*t_local + s) -> L[(t*ns + s)*128 + p]
            L = arr[tt].transpose(0, 2, 1).reshape(-1)  # [(t s p)]
            c0, n = SEG[name + str(pair)]
            idxg[:, c0 : c0 + n // 16] = pack_idx(L)
    return idxg, idxd


def unshard_output(outs):
    parts = []
    for o in outs:
        parts.append(
            np.asarray(o, np.float32)
            .reshape(P, TILES, N_NOISE)
            .transpose(1, 0, 2)
            .reshape(BPC, N_NOISE)
        )
    return np.concatenate(parts, axis=0)


def _install_profile_hook():
    import types

    if "antenv.axon_hooks" in sys.modules:
        return
    import antenv
    from trn_agent_boot.trn_boot import _ntff_profile_via_ctypes

    mod = types.ModuleType("antenv.axon_hooks")
    _state = {"hook": _ntff_profile_via_ctypes("/opt/axon/libaxon_pjrt.so")}
    mod.set_axon_ntff_profile_hook = lambda h: _state.__setitem__("hook", h)
    mod.get_axon_ntff_profile_hook = lambda: _state["hook"]
    sys.modules["antenv.axon_hooks"] = mod
    antenv.axon_hooks = mod


def kernel(context_ids, doc_ids, target_noise_ids, D, W, O, _trace=False):
    if _trace:
        _install_profile_hook()
    nc = get_nc()
    tbl = make_table(D, W, O)
    in_maps = []
    for c in range(N_CORES):
        idxg, idxd = make_core_inputs(context_ids, doc_ids, target_noise_ids, c)
        in_maps.append({"tbl": tbl, "idxg": idxg, "idxd": idxd})
    res = run_bass_kernel_spmd(
        nc, in_maps, core_ids=list(range(N_CORES)), trace=_trace
    )
    scores = unshard_output([res.results[c]["out"] for c in range(N_CORES)])
    if _trace:
        kernel.last_exec_time_ns = res.exec_time_ns
        kernel.last_results = res
    return scores


# revision 7
# speedup vs baseline: 1.3542x; 1.0705x over previous
"""Doc2vec-style embedding lookup + negative-sampling scores on 8 trn2 cores.

reference:
    x[b, :] = D[doc_ids[b]] + sum_c W[context_ids[b, c]]      # (B, 256)
    scores[b, k] = dot(x[b], O[:, target_noise_ids[b, k]])    # (B, 6)

Strategy (v2): data-parallel over batch (512 items/core), tables replicated
in bf16.  The baseline issued 60 indirect DMAs per core; each costs ~1.4us
of serialized Q7 SWDGE descriptor generation (994ns fixed + 0.34ns/desc),
so the kernel was Q7-bound at ~102us.  This version uses InstDMAGatherAnt
(dma_gather, mlp library), which amortizes the 994ns fixed cost over
thousands of descriptors.

dma_gather indices are int16 (sign-extended by the Q7; negatives fatal
mid-list), so a single gather can only span 32768 rows.  W (50000 rows) and
O^T (50000) are split into two 25001-row windows, each ending in an all-zero
row.  Every item gets 8 ctx slots in BOTH windows: real (window-relative)
ids fill slots in the window that owns them, remaining slots point at the
zero row.  Summing all 16 gathered rows equals the real 8-row ctx sum
(ctx slots are exchangeable under +).  Noise cols use the same trick per
slot: col = lo_gather[k] + hi_gather[k] (one is the real row, one is zero).
Doc rows (100000, int32 indices) stay on the exact indirect-DMA path.

Engines: SWDGE gathers (GpSimd) -> DVE folds lo+hi, 8-slot reduce, doc add,
noise mult -> ACT does the 6 per-slot dot-product accumulations
(activation accum_out = per-partition sum over free dim).
"""

import sys

sys.path.insert(0, "/opt/trn_rl_repo")

from contextlib import ExitStack

import ml_dtypes
import numpy as np

from concourse import bacc, bass, mybir
from concourse.bass_utils import run_bass_kernel_spmd
from concourse.library_config import mlp

VEC = 256
N_DOCS = 100000
N_WORDS = 50000
B = 4096
N_CTX = 8
N_NOISE = 6
N_CORES = 8
BPC = B // N_CORES  # 512
P = 128
TILES = BPC // P  # 4
WIN = 25000  # rows per gather window (zero row at local index WIN)
WROWS = WIN + 1
# table row layout (bf16, 256 wide)
W_LO = N_DOCS
W_HI = W_LO + WROWS
O_LO = W_HI + WROWS
O_HI = O_LO + WROWS
T_ROWS = O_HI + WROWS  # 200004

BF16 = mybir.dt.bfloat16

# idxg column layout (int16, 16-wrapped, replicated x8):
# cols per segment = num_idxs // 16
SEG = {}
_c = 0
for _name, _n in [
    ("wlo0", 2048), ("whi0", 2048), ("olo0", 1536), ("ohi0", 1536),
    ("wlo1", 2048), ("whi1", 2048), ("olo1", 1536), ("ohi1", 1536),
]:
    SEG[_name] = (_c, _n)
    _c += _n // 16
IDX_COLS = _c  # 896

_nc_cache = None


def build_nc():
    nc = bacc.Bacc(None, target_bir_lowering=False, debug=False, num_swdge_queues=4, dynamic_dma_scratch_size=65536)
    tbl = nc.declare_dram_parameter("tbl", [T_ROWS, VEC], BF16, isOutput=False)
    idxg = nc.declare_dram_parameter("idxg", [P, IDX_COLS], mybir.dt.int16, isOutput=False)
    idxd = nc.declare_dram_parameter("idxd", [P, TILES], mybir.dt.int32, isOutput=False)
    out = nc.declare_dram_parameter("out", [P, TILES * N_NOISE], mybir.dt.float32, isOutput=True)

    with ExitStack() as ctx:
        block = ctx.enter_context(nc.Block(no_gpsimd_drain=True))
        sem_idx = ctx.enter_context(nc.semaphore("sem_idx"))
        semW = [ctx.enter_context(nc.semaphore(f"semW{i}")) for i in range(2)]
        semO = [ctx.enter_context(nc.semaphore(f"semO{i}")) for i in range(2)]
        semD = ctx.enter_context(nc.semaphore("semD"))
        sem_prod = ctx.enter_context(nc.semaphore("sem_prod"))
        sem_act = ctx.enter_context(nc.semaphore("sem_act"))
        sem_out = ctx.enter_context(nc.semaphore("sem_out"))

        idxg_t = ctx.enter_context(nc.sbuf_tensor("idxg_t", [P, IDX_COLS], mybir.dt.int16))
        idxd_t = ctx.enter_context(nc.sbuf_tensor("idxd_t", [P, TILES], mybir.dt.int32))
        bufW = ctx.enter_context(nc.sbuf_tensor("bufW", [P, 64 * VEC], BF16))
        bufO = ctx.enter_context(nc.sbuf_tensor("bufO", [P, 48 * VEC], BF16))
        bufD = ctx.enter_context(nc.sbuf_tensor("bufD", [P, TILES * VEC], BF16))
        w8 = ctx.enter_context(nc.sbuf_tensor("w8", [P, 8 * VEC], BF16))
        xa = ctx.enter_context(nc.sbuf_tensor("xa", [P, VEC], BF16))
        xb = ctx.enter_context(nc.sbuf_tensor("xb", [P, VEC], BF16))
        cols = ctx.enter_context(nc.sbuf_tensor("cols", [P, N_NOISE * VEC], BF16))
        prod2 = ctx.enter_context(nc.sbuf_tensor("prod2", [P, 2 * N_NOISE * VEC], BF16))
        dump = ctx.enter_context(nc.sbuf_tensor("dump", [P, VEC], BF16))
        score_t = ctx.enter_context(nc.sbuf_tensor("score_t", [P, TILES * N_NOISE], mybir.dt.float32))

        # bufW slots: [pair*32 + 0:16) = lo (t_even s0-7, t_odd s0-7), +16 = hi
        # bufO slots: [pair*24 + 0:12) = lo (t_even k0-5, t_odd k0-5), +12 = hi

        @block.sync
        def _(s: bass.BassEngine):
            s.dma_start(out=idxg_t[:, :], in_=idxg[:, :]).then_inc(sem_idx, 16)
            s.dma_start(out=idxd_t[:, :], in_=idxd[:, :]).then_inc(sem_idx, 16)
            s.wait_ge(sem_act, TILES)
            s.dma_start(out=out[:, :], in_=score_t[:, :]).then_inc(sem_out, 16)
            s.wait_ge(sem_out, 16)

        @block.gpsimd
        def _(g: bass.BassGpSimd):
            g.load_library(mlp)
            g.wait_ge(sem_idx, 32)

            qctr = [0]

            def gather(seg, base, buf, slot0, nslots, sem):
                c0, n = SEG[seg]
                q = qctr[0] % 4
                qctr[0] += 1
                g.dma_gather(
                    out_ap=buf[:, slot0 * VEC : (slot0 + nslots) * VEC].rearrange(
                        "p (j d) -> p j d", j=nslots
                    ),
                    in_ap=tbl[base : base + WROWS, :],
                    idxs_ap=idxg_t[:, c0 : c0 + n // 16],
                    num_idxs=n,
                    num_idxs_reg=n,
                    elem_size=VEC,
                    single_packet=False,
                    queue_num=q,
                ).then_inc(sem, 16)

            for pair in range(2):
                sfx = str(pair)
                gather("wlo" + sfx, W_LO, bufW, pair * 32, 16, semW[pair])
                gather("whi" + sfx, W_HI, bufW, pair * 32 + 16, 16, semW[pair])
                for t in (2 * pair, 2 * pair + 1):
                    g.indirect_dma_start(
                        out=bufD[:, t * VEC : (t + 1) * VEC],
                        out_offset=None,
                        in_=tbl[:],
                        in_offset=bass.IndirectOffsetOnAxis(
                            ap=idxd_t[:, t : t + 1], axis=0
                        ),
                    ).then_inc(semD, 16)
                gather("olo" + sfx, O_LO, bufO, pair * 24, 12, semO[pair])
                gather("ohi" + sfx, O_HI, bufO, pair * 24 + 12, 12, semO[pair])

        @block.vector
        def _(v: bass.BassVectorEngine):
            with nc.allow_low_precision(reason="bf16 x/prod intermediates, f32 final accum"):
                for t in range(TILES):
                    pair, par = t // 2, t % 2
                    wlo0 = (pair * 32 + par * 8) * VEC
                    whi0 = (pair * 32 + 16 + par * 8) * VEC
                    v.wait_ge(semW[pair], 32)
                    v.tensor_tensor(
                        out=w8[:, :],
                        in0=bufW[:, wlo0 : wlo0 + 8 * VEC],
                        in1=bufW[:, whi0 : whi0 + 8 * VEC],
                        op=mybir.AluOpType.add,
                    )
                    v.tensor_reduce(
                        out=xa[:, :],
                        in_=w8[:, :].rearrange("p (s d) -> p d s", s=8),
                        axis=mybir.AxisListType.X,
                        op=mybir.AluOpType.add,
                    )
                    v.wait_ge(semD, 16 * (t + 1))
                    v.tensor_tensor(
                        out=xb[:, :],
                        in0=xa[:, :],
                        in1=bufD[:, t * VEC : (t + 1) * VEC],
                        op=mybir.AluOpType.add,
                    )
                    olo0 = (pair * 24 + par * 6) * VEC
                    ohi0 = (pair * 24 + 12 + par * 6) * VEC
                    v.wait_ge(semO[pair], 32)
                    if t >= 2:
                        v.wait_ge(sem_act, t - 1)  # prod2 slot t%2 free
                    v.tensor_tensor(
                        out=cols[:, :],
                        in0=bufO[:, olo0 : olo0 + 6 * VEC],
                        in1=bufO[:, ohi0 : ohi0 + 6 * VEC],
                        op=mybir.AluOpType.add,
                    )
                    pr = prod2[:, par * 6 * VEC : (par + 1) * 6 * VEC]
                    v.tensor_tensor(
                        out=pr.rearrange("p (k d) -> p k d", k=N_NOISE),
                        in0=xb[:, None, :].to_broadcast([P, N_NOISE, VEC]),
                        in1=cols[:, :].rearrange("p (k d) -> p k d", k=N_NOISE),
                        op=mybir.AluOpType.mult,
                    ).then_inc(sem_prod, 1)

        @block.scalar
        def _(a: bass.BassScalarEngine):
            for t in range(TILES):
                par = t % 2
                a.wait_ge(sem_prod, t + 1)
                for k in range(N_NOISE):
                    ins = a.activation(
                        out=dump[:, :],
                        in_=prod2[:, (par * 6 + k) * VEC : (par * 6 + k + 1) * VEC],
                        func=mybir.ActivationFunctionType.Copy,
                        accum_out=score_t[:, t * N_NOISE + k : t * N_NOISE + k + 1],
                    )
                ins.then_inc(sem_act, 1)

    nc.compile()
    return nc


def get_nc():
    global _nc_cache
    if _nc_cache is None:
        _nc_cache = build_nc()
    return _nc_cache


def make_table(D, W, O):
    """bf16 table [200004, 256]: D; Wlo; z; Whi; z; Olo; z; Ohi; z."""
    bf = ml_dtypes.bfloat16
    tbl = np.zeros((T_ROWS, VEC), dtype=bf)
    tbl[:N_DOCS] = np.asarray(D, np.float32).astype(bf)
    Wb = np.asarray(W, np.float32).astype(bf)
    tbl[W_LO : W_LO + WIN] = Wb[:WIN]
    tbl[W_HI : W_HI + WIN] = Wb[WIN:]
    Ob = np.ascontiguousarray(np.asarray(O, np.float32).T).astype(bf)
    tbl[O_LO : O_LO + WIN] = Ob[:WIN]
    tbl[O_HI : O_HI + WIN] = Ob[WIN:]
    return tbl


def pack_idx(L):
    """list of n int idxs -> [128, n//16] int16 (16-wrapped, replicated x8)."""
    n = L.shape[0]
    A = L.reshape(n // 16, 16).T.astype(np.int16)  # [16, n//16]
    return np.tile(A, (8, 1))


def make_core_inputs(context_ids, doc_ids, target_noise_ids, core):
    """Returns (idxg [128, IDX_COLS] i16, idxd [128, TILES] i32)."""
    sl = slice(core * BPC, (core + 1) * BPC)
    ctx = np.asarray(context_ids, np.int64)[sl].reshape(TILES, P, N_CTX)
    doc = np.asarray(doc_ids, np.int64)[sl].reshape(TILES, P)
    noi = np.asarray(target_noise_ids, np.int64)[sl].reshape(TILES, P, N_NOISE)

    idxd = doc.T.astype(np.int32).copy()  # [128, TILES]

    BIG = 1 << 20
    lo = np.sort(np.where(ctx < WIN, ctx, BIG), axis=-1)
    lo = np.where(lo >= BIG, WIN, lo)  # [T, P, 8]
    hi = np.sort(np.where(ctx >= WIN, ctx - WIN, BIG), axis=-1)
    hi = np.where(hi >= BIG, WIN, hi)
    nlo = np.where(noi < WIN, noi, WIN)  # [T, P, 6]
    nhi = np.where(noi >= WIN, noi - WIN, WIN)

    idxg = np.empty((P, IDX_COLS), dtype=np.int16)
    for pair in range(2):
        tt = slice(2 * pair, 2 * pair + 2)
        for name, arr in [("wlo", lo), ("whi", hi), ("olo", nlo), ("ohi", nhi)]:
            # positions i = p + 128*(nslots*t_local + s) -> L[(t*ns + s)*128 + p]
            L = arr[tt].transpose(0, 2, 1).reshape(-1)  # [(t s p)]
            c0, n = SEG[name + str(pair)]
            idxg[:, c0 : c0 + n // 16] = pack_idx(L)
    return idxg, idxd


def unshard_output(outs):
    parts = []
    for o in outs:
        parts.append(
            np.asarray(o, np.float32)
            .reshape(P, TILES, N_NOISE)
            .transpose(1, 0, 2)
            .reshape(BPC, N_NOISE)
        )
    return np.concatenate(parts, axis=0)


def _install_profile_hook():
    import types

    if "antenv.axon_hooks" in sys.modules:
        return
    import antenv
    from trn_agent_boot.trn_boot import _ntff_profile_via_ctypes

    mod = types.ModuleType("antenv.axon_hooks")
    _state = {"hook": _ntff_profile_via_ctypes("/opt/axon/libaxon_pjrt.so")}
    mod.set_axon_ntff_profile_hook = lambda h: _state.__setitem__("hook", h)
    mod.get_axon_ntff_profile_hook = lambda: _state["hook"]
    sys.modules["antenv.axon_hooks"] = mod
    antenv.axon_hooks = mod


def kernel(context_ids, doc_ids, target_noise_ids, D, W, O, _trace=False):
    if _trace:
        _install_profile_hook()
    nc = get_nc()
    tbl = make_table(D, W, O)
    in_maps = []
    for c in range(N_CORES):
        idxg, idxd = make_core_inputs(context_ids, doc_ids, target_noise_ids, c)
        in_maps.append({"tbl": tbl, "idxg": idxg, "idxd": idxd})
    res = run_bass_kernel_spmd(
        nc, in_maps, core_ids=list(range(N_CORES)), trace=_trace
    )
    scores = unshard_output([res.results[c]["out"] for c in range(N_CORES)])
    if _trace:
        kernel.last_exec_time_ns = res.exec_time_ns
        kernel.last_results = res
    return scores


# revision 9
# speedup vs baseline: 1.5881x; 1.1727x over previous
"""Doc2vec-style embedding lookup + negative-sampling scores on 8 trn2 cores.

reference:
    x[b, :] = D[doc_ids[b]] + sum_c W[context_ids[b, c]]      # (B, 256)
    scores[b, k] = dot(x[b], O[:, target_noise_ids[b, k]])    # (B, 6)

Strategy (v2): data-parallel over batch (512 items/core), tables replicated
in bf16.  The baseline issued 60 indirect DMAs per core; each costs ~1.4us
of serialized Q7 SWDGE descriptor generation (994ns fixed + 0.34ns/desc),
so the kernel was Q7-bound at ~102us.  This version uses InstDMAGatherAnt
(dma_gather, mlp library), which amortizes the 994ns fixed cost over
thousands of descriptors.

dma_gather indices are int16 (sign-extended by the Q7; negatives fatal
mid-list), so a single gather can only span 32768 rows.  W (50000 rows) and
O^T (50000) are split into two 25001-row windows, each ending in an all-zero
row.  Every item gets 8 ctx slots in BOTH windows: real (window-relative)
ids fill slots in the window that owns them, remaining slots point at the
zero row.  Summing all 16 gathered rows equals the real 8-row ctx sum
(ctx slots are exchangeable under +).  Noise cols use the same trick per
slot: col = lo_gather[k] + hi_gather[k] (one is the real row, one is zero).
Doc rows (100000, int32 indices) stay on the exact indirect-DMA path.

Engines: SWDGE gathers (GpSimd) -> DVE folds lo+hi, 8-slot reduce, doc add,
noise mult -> ACT does the 6 per-slot dot-product accumulations
(activation accum_out = per-partition sum over free dim).
"""

import sys

sys.path.insert(0, "/opt/trn_rl_repo")

from contextlib import ExitStack

import ml_dtypes
import numpy as np

from concourse import bacc, bass, mybir
from concourse.bass_utils import run_bass_kernel_spmd
from concourse.library_config import mlp

VEC = 256
N_DOCS = 100000
N_WORDS = 50000
B = 4096
N_CTX = 8
N_NOISE = 6
N_CORES = 8
BPC = B // N_CORES  # 512
P = 128
TILES = BPC // P  # 4
WIN = 25000  # rows per gather window (zero row at local index WIN)
WROWS = WIN + 1
# table row layout (bf16, 256 wide)
W_LO = N_DOCS
W_HI = W_LO + WROWS
O_LO = W_HI + WROWS
O_HI = O_LO + WROWS
T_ROWS = O_HI + WROWS  # 200004

BF16 = mybir.dt.bfloat16

# idxg column layout (int16, 16-wrapped, replicated x8):
# cols per segment = num_idxs // 16
SEG = {}
_c = 0
for _t in range(4):
    for _name, _n in [("wlo", 1024), ("whi", 1024), ("olo", 768), ("ohi", 768)]:
        SEG[_name + str(_t)] = (_c, _n)
        _c += _n // 16
IDX_COLS = _c  # 896

_nc_cache = None


def build_nc():
    nc = bacc.Bacc(None, target_bir_lowering=False, debug=False, num_swdge_queues=4, dynamic_dma_scratch_size=65536)
    tbl = nc.declare_dram_parameter("tbl", [T_ROWS, VEC], BF16, isOutput=False)
    idxg = nc.declare_dram_parameter("idxg", [P, IDX_COLS], mybir.dt.int16, isOutput=False)
    idxd = nc.declare_dram_parameter("idxd", [P, TILES], mybir.dt.int32, isOutput=False)
    out = nc.declare_dram_parameter("out", [P, TILES * N_NOISE], mybir.dt.float32, isOutput=True)

    with ExitStack() as ctx:
        block = ctx.enter_context(nc.Block(no_gpsimd_drain=True))
        sem_idx = ctx.enter_context(nc.semaphore("sem_idx"))
        semW = [ctx.enter_context(nc.semaphore(f"semW{i}")) for i in range(TILES)]
        semO = [ctx.enter_context(nc.semaphore(f"semO{i}")) for i in range(TILES)]
        semD = ctx.enter_context(nc.semaphore("semD"))
        sem_prod = ctx.enter_context(nc.semaphore("sem_prod"))
        sem_act = ctx.enter_context(nc.semaphore("sem_act"))
        sem_out = ctx.enter_context(nc.semaphore("sem_out"))

        idxg_t = ctx.enter_context(nc.sbuf_tensor("idxg_t", [P, IDX_COLS], mybir.dt.int16))
        idxd_t = ctx.enter_context(nc.sbuf_tensor("idxd_t", [P, TILES], mybir.dt.int32))
        bufW = ctx.enter_context(nc.sbuf_tensor("bufW", [P, 64 * VEC], BF16))
        bufO = ctx.enter_context(nc.sbuf_tensor("bufO", [P, 48 * VEC], BF16))
        bufD = ctx.enter_context(nc.sbuf_tensor("bufD", [P, TILES * VEC], BF16))
        w8 = ctx.enter_context(nc.sbuf_tensor("w8", [P, 8 * VEC], BF16))
        xa = ctx.enter_context(nc.sbuf_tensor("xa", [P, VEC], BF16))
        xb = ctx.enter_context(nc.sbuf_tensor("xb", [P, VEC], BF16))
        cols = ctx.enter_context(nc.sbuf_tensor("cols", [P, N_NOISE * VEC], BF16))
        prod2 = ctx.enter_context(nc.sbuf_tensor("prod2", [P, 2 * N_NOISE * VEC], BF16))
        dump = ctx.enter_context(nc.sbuf_tensor("dump", [P, VEC], BF16))
        score_t = ctx.enter_context(nc.sbuf_tensor("score_t", [P, TILES * N_NOISE], mybir.dt.float32))

        # bufW slots: [pair*32 + 0:16) = lo (t_even s0-7, t_odd s0-7), +16 = hi
        # bufO slots: [pair*24 + 0:12) = lo (t_even k0-5, t_odd k0-5), +12 = hi

        @block.sync
        def _(s: bass.BassEngine):
            s.dma_start(out=idxg_t[:, :], in_=idxg[:, :]).then_inc(sem_idx, 16)
            s.dma_start(out=idxd_t[:, :], in_=idxd[:, :]).then_inc(sem_idx, 16)
            s.wait_ge(sem_act, TILES)
            s.dma_start(out=out[:, :], in_=score_t[:, :]).then_inc(sem_out, 16)
            s.wait_ge(sem_out, 16)

        @block.gpsimd
        def _(g: bass.BassGpSimd):
            g.load_library(mlp)
            g.wait_ge(sem_idx, 32)
            r1024 = g.to_reg(1024)
            r768 = g.to_reg(768)

            def gather(seg, base, buf, slot0, nslots, sem, q, reg):
                c0, n = SEG[seg]
                g.dma_gather(
                    out_ap=buf[:, slot0 * VEC : (slot0 + nslots) * VEC].rearrange(
                        "p (j d) -> p j d", j=nslots
                    ),
                    in_ap=tbl[base : base + WROWS, :],
                    idxs_ap=idxg_t[:, c0 : c0 + n // 16],
                    num_idxs=n,
                    num_idxs_reg=reg,
                    elem_size=VEC,
                    single_packet=False,
                    queue_num=q,
                ).then_inc(sem, 16)

            # doc rows first (small, exact; fixed queue 0)
            for t in range(TILES):
                g.indirect_dma_start(
                    out=bufD[:, t * VEC : (t + 1) * VEC],
                    out_offset=None,
                    in_=tbl[:],
                    in_offset=bass.IndirectOffsetOnAxis(
                        ap=idxd_t[:, t : t + 1], axis=0
                    ),
                ).then_inc(semD, 16)
            # per-tile gathers, one gather of each kind per queue, tile order
            # bufW slots: tile t lo at [t*8, t*8+8), hi at [32 + t*8, ...)
            # bufO slots: tile t lo at [t*6, t*6+6), hi at [24 + t*6, ...)
            for t in range(TILES):
                sfx = str(t)
                gather("wlo" + sfx, W_LO, bufW, t * 8, 8, semW[t], t % 4, r1024)
                gather("whi" + sfx, W_HI, bufW, 32 + t * 8, 8, semW[t], (t + 1) % 4, r1024)
                gather("olo" + sfx, O_LO, bufO, t * 6, 6, semO[t], (t + 2) % 4, r768)
                gather("ohi" + sfx, O_HI, bufO, 24 + t * 6, 6, semO[t], (t + 3) % 4, r768)

        @block.vector
        def _(v: bass.BassVectorEngine):
            with nc.allow_low_precision(reason="bf16 x/prod intermediates, f32 final accum"):
                for t in range(TILES):
                    par = t % 2
                    wlo0 = (t * 8) * VEC
                    whi0 = (32 + t * 8) * VEC
                    v.wait_ge(semW[t], 32)
                    v.tensor_tensor(
                        out=w8[:, :],
                        in0=bufW[:, wlo0 : wlo0 + 8 * VEC],
                        in1=bufW[:, whi0 : whi0 + 8 * VEC],
                        op=mybir.AluOpType.add,
                    )
                    v.tensor_reduce(
                        out=xa[:, :],
                        in_=w8[:, :].rearrange("p (s d) -> p d s", s=8),
                        axis=mybir.AxisListType.X,
                        op=mybir.AluOpType.add,
                    )
                    v.wait_ge(semD, 16 * (t + 1))
                    v.tensor_tensor(
                        out=xb[:, :],
                        in0=xa[:, :],
                        in1=bufD[:, t * VEC : (t + 1) * VEC],
                        op=mybir.AluOpType.add,
                    )
                    olo0 = (t * 6) * VEC
                    ohi0 = (24 + t * 6) * VEC
                    v.wait_ge(semO[t], 32)
                    if t >= 2:
                        v.wait_ge(sem_act, t - 1)  # prod2 slot t%2 free
                    v.tensor_tensor(
                        out=cols[:, :],
                        in0=bufO[:, olo0 : olo0 + 6 * VEC],
                        in1=bufO[:, ohi0 : ohi0 + 6 * VEC],
                        op=mybir.AluOpType.add,
                    )
                    pr = prod2[:, par * 6 * VEC : (par + 1) * 6 * VEC]
                    v.tensor_tensor(
                        out=pr.rearrange("p (k d) -> p k d", k=N_NOISE),
                        in0=xb[:, None, :].to_broadcast([P, N_NOISE, VEC]),
                        in1=cols[:, :].rearrange("p (k d) -> p k d", k=N_NOISE),
                        op=mybir.AluOpType.mult,
                    ).then_inc(sem_prod, 1)

        @block.scalar
        def _(a: bass.BassScalarEngine):
            for t in range(TILES):
                par = t % 2
                a.wait_ge(sem_prod, t + 1)
                for k in range(N_NOISE):
                    ins = a.activation(
                        out=dump[:, :],
                        in_=prod2[:, (par * 6 + k) * VEC : (par * 6 + k + 1) * VEC],
                        func=mybir.ActivationFunctionType.Copy,
                        accum_out=score_t[:, t * N_NOISE + k : t * N_NOISE + k + 1],
                    )
                ins.then_inc(sem_act, 1)

    nc.compile()
    return nc


def get_nc():
    global _nc_cache
    if _nc_cache is None:
        _nc_cache = build_nc()
    return _nc_cache


def make_table(D, W, O):
    """bf16 table [200004, 256]: D; Wlo; z; Whi; z; Olo; z; Ohi; z."""
    bf = ml_dtypes.bfloat16
    tbl = np.zeros((T_ROWS, VEC), dtype=bf)
    tbl[:N_DOCS] = np.asarray(D, np.float32).astype(bf)
    Wb = np.asarray(W, np.float32).astype(bf)
    tbl[W_LO : W_LO + WIN] = Wb[:WIN]
    tbl[W_HI : W_HI + WIN] = Wb[WIN:]
    Ob = np.ascontiguousarray(np.asarray(O, np.float32).T).astype(bf)
    tbl[O_LO : O_LO + WIN] = Ob[:WIN]
    tbl[O_HI : O_HI + WIN] = Ob[WIN:]
    return tbl


def pack_idx(L):
    """list of n int idxs -> [128, n//16] int16 (16-wrapped, replicated x8)."""
    n = L.shape[0]
    A = L.reshape(n // 16, 16).T.astype(np.int16)  # [16, n//16]
    return np.tile(A, (8, 1))


def make_core_inputs(context_ids, doc_ids, target_noise_ids, core):
    """Returns (idxg [128, IDX_COLS] i16, idxd [128, TILES] i32)."""
    sl = slice(core * BPC, (core + 1) * BPC)
    ctx = np.asarray(context_ids, np.int64)[sl].reshape(TILES, P, N_CTX)
    doc = np.asarray(doc_ids, np.int64)[sl].reshape(TILES, P)
    noi = np.asarray(target_noise_ids, np.int64)[sl].reshape(TILES, P, N_NOISE)

    idxd = doc.T.astype(np.int32).copy()  # [128, TILES]

    BIG = 1 << 20
    lo = np.sort(np.where(ctx < WIN, ctx, BIG), axis=-1)
    lo = np.where(lo >= BIG, WIN, lo)  # [T, P, 8]
    hi = np.sort(np.where(ctx >= WIN, ctx - WIN, BIG), axis=-1)
    hi = np.where(hi >= BIG, WIN, hi)
    nlo = np.where(noi < WIN, noi, WIN)  # [T, P, 6]
    nhi = np.where(noi >= WIN, noi - WIN, WIN)

    idxg = np.empty((P, IDX_COLS), dtype=np.int16)
    for t in range(TILES):
        for name, arr in [("wlo", lo), ("whi", hi), ("olo", nlo), ("ohi", nhi)]:
            # positions i = p + 128*s -> L[s*128 + p]
            L = arr[t].T.reshape(-1)  # [(s p)]
            c0, n = SEG[name + str(t)]
            idxg[:, c0 : c0 + n // 16] = pack_idx(L)
    return idxg, idxd


def unshard_output(outs):
    parts = []
    for o in outs:
        parts.append(
            np.asarray(o, np.float32)
            .reshape(P, TILES, N_NOISE)
            .transpose(1, 0, 2)
            .reshape(BPC, N_NOISE)
        )
    return np.concatenate(parts, axis=0)


def _install_profile_hook():
    import types

    if "antenv.axon_hooks" in sys.modules:
        return
    import antenv
    from trn_agent_boot.trn_boot import _ntff_profile_via_ctypes

    mod = types.ModuleType("antenv.axon_hooks")
    _state = {"hook": _ntff_profile_via_ctypes("/opt/axon/libaxon_pjrt.so")}
    mod.set_axon_ntff_profile_hook = lambda h: _state.__setitem__("hook", h)
    mod.get_axon_ntff_profile_hook = lambda: _state["hook"]
    sys.modules["antenv.axon_hooks"] = mod
    antenv.axon_hooks = mod


def kernel(context_ids, doc_ids, target_noise_ids, D, W, O, _trace=False):
    if _trace:
        _install_profile_hook()
    nc = get_nc()
    tbl = make_table(D, W, O)
    in_maps = []
    for c in range(N_CORES):
        idxg, idxd = make_core_inputs(context_ids, doc_ids, target_noise_ids, c)
        in_maps.append({"tbl": tbl, "idxg": idxg, "idxd": idxd})
    res = run_bass_kernel_spmd(
        nc, in_maps, core_ids=list(range(N_CORES)), trace=_trace
    )
    scores = unshard_output([res.results[c]["out"] for c in range(N_CORES)])
    if _trace:
        kernel.last_exec_time_ns = res.exec_time_ns
        kernel.last_results = res
    return scores
